# revision 39
# baseline (speedup 1.0000x reference)
"""Multi-head attention (B=4, S=2048, D=1024, H=16) on 8 Trainium2 cores.

Sharding: core c = (batch b = c//2, head-group g = c%2). Each core computes
8 heads' attention for one batch element plus the partial output projection
for its head-group's rows of Wo; the host sums the two partials per batch
and adds the bias.

Per-core kernel (all matmuls bf16, fp32 accumulation):
  xT      = host-transposed cast_bf16(x)                [D, S]
  qT, kT  = Wg.T @ x.T (lhsT = W chunks, rhs = xT)      [G, S]
  v       = x @ Wv     (lhsT = xT chunks, rhs = Wv)     [S, G]
  per head-pair, per 512-wide q window, per key-block kb:
    sT    = k_h @ q_h.T (row-paired heads, K=64; the two heads load
            different PE row-groups and stream concurrently)  PSUM [128,1024]
    pT    = exp(sT / 8) on ScalarE -> bf16 SBUF
    ctx+den fused: stationary [v_h | 1] (M=65) so the softmax denominator
      accumulates in PSUM row 64 of the SAME matmul — matmul cost is N
      cycles regardless of M, so the denominator is free.
  norm    : den rows -> bf16 SBUF, K=1 ones-matmuls broadcast them across
            lanes 0:64, one reciprocal + multiply per head. Head B's
            product lane-shifts to ctxT partitions 64:128 via SBUF->SBUF
            DMA (DMA crosses partitions; DVE cannot).
  out     = ctxT.T @ Wo_g  -> fp32 partial to DRAM      [S, D]

Schedule notes. The exp stream paces the attention loop at ~1.1us per key
block; the PE's own attention work is only ~0.65us of that, so every other
matmul (projections, out-projection, normalization broadcasts) must WEAVE
INTO the attention stream rather than bunch up — a bunched phase leaves the
ACT engine idle and that idle time is unrecoverable (measured versions that
serialized projections at pair boundaries lost ~20us per boundary, plus a
~3us half-clock p-state ramp after every PE idle gap):
  - pair-major loop order: pair p runs q-windows 0..3 back-to-back, and
    pair p+1's Q/K projection drips through those windows as background
    work, a couple of matmuls per key block, from a dedicated PSUM bank
    (tag "proj").
  - normalization for window w is deferred into window w+1's kb=1 slot so
    its broadcast matmuls never head-of-line block the in-order PE queue
    (their DVE den-copy inputs are ready by then). The broadcast pair
    shares one rotation slot with the score tiles (tag "pscore").
  - ctx trails scores by two key blocks; the third pctx buffer lets the
    next window's first ctx start before the deferred norm drains.
  - out-projection of window w drips through pair 3's window w+1 (it
    needs all four pairs' ctxT columns); only window 3's out-projection
    remains as the kernel tail.
"""

import numpy as np

B, S, D = 4, 2048, 1024
H, HD = 16, 64
NCORES = 8
G = D // 2  # head-group width per core (8 heads x 64)

_BUILD_CACHE = {}


def build_mha(S=S, D=D, G=G, HD=HD):
    """Build the per-core Bass program. Returns the Bass object."""
    key = (S, D, G, HD)
    if key in _BUILD_CACHE:
        return _BUILD_CACHE[key]

    import concourse.bacc as bacc
    import concourse.mybir as mybir
    import concourse.tile as tile
    from contextlib import ExitStack

    FP32 = mybir.dt.float32
    BF16 = mybir.dt.bfloat16

    P = 128
    DC = D // P          # d_in chunks
    GC = G // P          # head-pair chunks
    SB = S // P          # seq blocks (key blocks)
    W = 512              # q window width
    NW = S // W          # number of q windows
    VW = HD + 1          # per-head v slab: [v_h 64 | 1]
    assert G % P == 0 and HD == 64 and S % 512 == 0

    nc = bacc.Bacc("TRN2", target_bir_lowering=False, debug=False)
    xt_d = nc.declare_dram_parameter("xt", [D, S], BF16, isOutput=False)
    wq_d = nc.declare_dram_parameter("wq", [D, G], BF16, isOutput=False)
    wk_d = nc.declare_dram_parameter("wk", [D, G], BF16, isOutput=False)
    wv_d = nc.declare_dram_parameter("wv", [D, G], BF16, isOutput=False)
    wo_d = nc.declare_dram_parameter("wo", [G, D], BF16, isOutput=False)
    out_d = nc.declare_dram_parameter("out", [S, D], FP32, isOutput=True)

    with tile.TileContext(nc) as tc, ExitStack() as ctx:
        const = ctx.enter_context(tc.tile_pool(name="const", bufs=1))
        wpool = ctx.enter_context(tc.tile_pool(name="wpool", bufs=1))
        big = ctx.enter_context(tc.tile_pool(name="big", bufs=1))
        ppool = ctx.enter_context(tc.tile_pool(name="ppool", bufs=4))
        norm = ctx.enter_context(tc.tile_pool(name="norm", bufs=4))
        outp = ctx.enter_context(tc.tile_pool(name="outp", bufs=6))
        pscore = ctx.enter_context(tc.tile_pool(name="pscore", bufs=2, space="PSUM"))
        pctx = ctx.enter_context(tc.tile_pool(name="pctx", bufs=3, space="PSUM"))
        pproj = ctx.enter_context(tc.tile_pool(name="pproj", bufs=1, space="PSUM"))

        zbias = const.tile([P, 1], FP32)
        nc.gpsimd.memset(zbias[:], 0.0)
        ones_sb = const.tile([P, HD], BF16)
        nc.gpsimd.memset(ones_sb[:], 1.0)

        # ---- loads ----
        # Ordered so the first projection's operands land first (wq + the
        # sc=0 column block of xt, then wk/xt1, ...), split across the two
        # HWDGE trigger engines (Sync + Activation; ACT is idle at start).
        wq_sb = wpool.tile([P, DC, G], BF16)
        wk_sb = wpool.tile([P, DC, G], BF16)
        wv_sb = wpool.tile([P, DC, G], BF16)
        wo_sb = wpool.tile([P, GC, D], BF16)
        xt = big.tile([P, DC, S], BF16)

        # weights and most of xt stream on the Sync trigger engine in
        # criticality order: pair-0's wq/wk column slices and wv early
        # (the first window needs them), the other pairs' weight columns
        # and wo late (their background projections run tens of us in).
        # The ACT engine's queue must stay nearly clear of DMA triggers —
        # trigger instructions there head-of-line block the first exps
        # behind the input stream's ring-full waits (measured: first exp
        # at 42us with wait=0, purely queue-ordering).
        def xt_dma(eng, dc, sc):
            eng.dma_start(
                xt[:, dc, sc * 512:(sc + 1) * 512],
                xt_d[dc * P:(dc + 1) * P, sc * 512:(sc + 1) * 512],
            )

        # scalar (ACT) carries ONLY xt-sc0: its 8 triggers clear the ACT
        # queue by the time the first scores are ready; more would delay
        # the first exp behind DMA ring-full waits.
        for dc in range(DC):
            nc.sync.dma_start(wq_sb[:, dc, 0:P], wq_d[dc * P:(dc + 1) * P, 0:P])
        for dc in range(DC):
            xt_dma(nc.scalar, dc, 0)
        for dc in range(DC):
            nc.sync.dma_start(wk_sb[:, dc, 0:P], wk_d[dc * P:(dc + 1) * P, 0:P])
        for dc in range(DC):
            nc.sync.dma_start(wv_sb[:, dc, :], wv_d[dc * P:(dc + 1) * P, :])
        for sc in (1, 2, 3):
            for dc in range(DC):
                xt_dma(nc.sync, dc, sc)
        for dc in range(DC):
            nc.sync.dma_start(wq_sb[:, dc, P:G], wq_d[dc * P:(dc + 1) * P, P:G])
        for dc in range(DC):
            nc.sync.dma_start(wk_sb[:, dc, P:G], wk_d[dc * P:(dc + 1) * P, P:G])
        for dc in range(GC):
            nc.sync.dma_start(wo_sb[:, dc, :], wo_d[dc * P:(dc + 1) * P, :])

        # ---- projection helpers ----
        qt = big.tile([P, GC, S], BF16)
        kt = big.tile([P, GC, S], BF16)
        # v laid out per (seq-block, head) as a contiguous [v_h | 1] slab
        # of 65 columns; the memset supplies the ones column.
        vp = big.tile([P, SB, 2 * GC, VW], BF16)
        # memset on DVE: gpsimd's queue must stay clear for the xt DMA
        # triggers above (a 7us memset there delays the xt stream)
        nc.vector.memset(vp[:], 1.0)

        def emit_proj_qk_one(g, which, sc):
            w_sb, dst = (wq_sb, qt) if which == "q" else (wk_sb, kt)
            ps = pproj.tile([P, 512], FP32, tag="proj", name="ps")
            for dc in range(DC):
                nc.tensor.matmul(
                    ps[:],
                    lhsT=w_sb[:, dc, g * P:(g + 1) * P],
                    rhs=xt[:, dc, sc * 512:(sc + 1) * 512],
                    start=(dc == 0),
                    stop=(dc == DC - 1),
                )
            nc.vector.tensor_copy(dst[:, g, sc * 512:(sc + 1) * 512], ps[:])

        def emit_proj_v(sb):
            ps = pproj.tile([P, 2 * GC, HD], FP32, tag="proj", name="ps")
            for dc in range(DC):
                nc.tensor.matmul(
                    ps[:, :, :],
                    lhsT=xt[:, dc, sb * P:(sb + 1) * P],
                    rhs=wv_sb[:, dc, :],
                    start=(dc == 0),
                    stop=(dc == DC - 1),
                )
            # one strided copy: [v cols only], dst stride 65 vs src 64
            nc.vector.tensor_copy(vp[:, sb, :, 0:HD], ps[:, :, :])

        # ---- background work queue ----
        # (emit_fn, is_matmul) pairs; the kb loop drains up to `budget`
        # matmuls per key block (copies/DMAs ride along free) so non-
        # attention PE work interleaves into the exp-paced stream instead
        # of bunching into an ACT-idle phase.
        bg = []

        def bg_step(budget):
            while bg and budget > 0:
                fn, is_mm = bg[0]
                if is_mm and budget <= 0:
                    break
                bg.pop(0)
                fn()
                if is_mm:
                    budget -= 1
            # drain any leading non-matmul thunks
            while bg and not bg[0][1]:
                bg.pop(0)[0]()

        def queue_proj_qk(g):
            for which in ("q", "k"):
                for sc in range(S // 512):
                    queue_one_qk(g, which, sc)

        def queue_outproj(w):
            # 8 tiles (4 row blocks x 2 column halves), each: 4 psum-
            # accumulating matmuls over the pairs, then copy + store.
            for t in range(8):
                sb, nck = divmod(t, 2)
                row = w * W + sb * P
                box = {}

                def mmfn(row=row, nck=nck, box=box):
                    g = box.setdefault("g", 0)
                    if g == 0:
                        box["po"] = pproj.tile(
                            [P, 512], FP32, tag="proj", name="po"
                        )
                    nc.tensor.matmul(
                        box["po"][:],
                        lhsT=ctxT[:, g, row:row + P],
                        rhs=wo_sb[:, g, nck * 512:(nck + 1) * 512],
                        start=(g == 0),
                        stop=(g == GC - 1),
                    )
                    box["g"] = g + 1

                for _ in range(GC):
                    bg.append((mmfn, True))

                def cpfn(row=row, nck=nck, box=box):
                    ob = outp.tile([P, 512], FP32, tag="ob")
                    nc.vector.tensor_copy(ob[:], box["po"][:])
                    nc.sync.dma_start(
                        out_d[row:row + P, nck * 512:(nck + 1) * 512], ob[:]
                    )

                bg.append((cpfn, False))

        # minimal prologue: just what window (0,0)'s first key blocks
        # need (q-sc0, k-sc0, v0) — everything else drips through the
        # window as background work, ordered by deadline: k-sc1/2/3 are
        # needed at key blocks 4/8/12 of EVERY pair-0 window, q-sc1/2/3
        # only at windows 1/2/3.
        emit_proj_qk_one(0, "q", 0)
        emit_proj_qk_one(0, "k", 0)
        emit_proj_v(0)

        def queue_one_qk(g, which, sc):
            box = {}

            def mmfn(g=g, which=which, sc=sc, box=box):
                dc = box.setdefault("dc", 0)
                if dc == 0:
                    box["ps"] = pproj.tile([P, 512], FP32, tag="proj", name="ps")
                w_sb = wq_sb if which == "q" else wk_sb
                nc.tensor.matmul(
                    box["ps"][:],
                    lhsT=w_sb[:, dc, g * P:(g + 1) * P],
                    rhs=xt[:, dc, sc * 512:(sc + 1) * 512],
                    start=(dc == 0),
                    stop=(dc == DC - 1),
                )
                box["dc"] = dc + 1

            for _ in range(DC):
                bg.append((mmfn, True))

            def cpfn(g=g, which=which, sc=sc, box=box):
                dst = qt if which == "q" else kt
                nc.vector.tensor_copy(
                    dst[:, g, sc * 512:(sc + 1) * 512], box["ps"][:]
                )

            bg.append((cpfn, False))



        # ---- attention ----
        ctxT = big.tile([P, GC, S], BF16)
        EXP = mybir.ActivationFunctionType.Exp
        scale = float(1.0 / np.sqrt(HD))

        def emit_norm(p, q0, cA, cB):
            den = norm.tile([P, 1024], BF16, tag="den")
            nc.vector.tensor_copy(den[64:65, 0:512], cA[64:65, :])
            nc.vector.tensor_copy(den[64:65, 512:1024], cB[64:65, :])
            bc = pscore.tile([P, 1024], FP32, tag="pscore", name="bc")
            nc.tensor.matmul(
                bc[0:64, 0:512], lhsT=ones_sb[64:65, :],
                rhs=den[64:65, 0:512], start=True, stop=True,
            )
            nc.tensor.matmul(
                bc[0:64, 512:1024], lhsT=ones_sb[64:65, :],
                rhs=den[64:65, 512:1024], start=True, stop=True,
            )
            rec = norm.tile([P, 1024], FP32, tag="rec")
            nc.vector.reciprocal_approx_fast(rec[0:64, 0:512], bc[0:64, 0:512])
            nc.vector.tensor_tensor(
                ctxT[0:64, p, q0:q0 + 512],
                cA[0:64, :],
                rec[0:64, 0:512],
                mybir.AluOpType.mult,
            )
            nc.vector.reciprocal_approx_fast(
                rec[0:64, 512:1024], bc[0:64, 512:1024]
            )
            tmpB = norm.tile([P, 512], BF16, tag="tmpB")
            nc.vector.tensor_tensor(
                tmpB[0:64, :],
                cB[0:64, :],
                rec[0:64, 512:1024],
                mybir.AluOpType.mult,
            )
            nc.sync.dma_start(ctxT[64:128, p, q0:q0 + 512], tmpB[0:64, :])

        pending_norm = None  # (p, q0, cA, cB) awaiting next window's kb=1

        for p in range(GC):
            hA, hB = 2 * p, 2 * p + 1
            for qw in range(NW):
                q0 = qw * W
                # next pair's projections drip through this pair's later
                # windows; the FIFO queue naturally holds them behind
                # pair 0's own q-sc2/3 leftovers, spreading the load over
                # all remaining windows instead of compressing it
                if p == 0 and qw == 1:
                    queue_one_qk(0, "q", 2)
                    queue_one_qk(0, "q", 3)
                if p < GC - 1 and qw == 1:
                    queue_proj_qk(p + 1)
                if p == GC - 1 and qw >= 1:
                    queue_outproj(qw - 1)  # needs all pairs at window qw-1
                cA = pctx.tile([P, 512], FP32, tag="pctx", name="cA")
                cB = pctx.tile([P, 512], FP32, tag="pctx", name="cB")

                def emit_scores_exp(kb):
                    s = pscore.tile([P, 1024], FP32, tag="pscore", name="s")
                    nc.tensor.matmul(
                        s[:, 0:512],
                        lhsT=kt[0:64, p, kb * P:(kb + 1) * P],
                        rhs=qt[0:64, p, q0:q0 + 512],
                        start=True, stop=True,
                    )
                    nc.tensor.matmul(
                        s[:, 512:1024],
                        lhsT=kt[64:128, p, kb * P:(kb + 1) * P],
                        rhs=qt[64:128, p, q0:q0 + 512],
                        start=True, stop=True,
                    )
                    pt = ppool.tile([P, 1024], BF16, tag="ppool", name="pt")
                    nc.scalar.activation(
                        pt[:], s[:], EXP, bias=zbias[:], scale=scale
                    )
                    return pt

                def emit_ctx(kb, pt):
                    first, last = kb == 0, kb == SB - 1
                    nc.tensor.matmul(
                        cA[0:65, :],
                        lhsT=vp[:, kb, hA, :],
                        rhs=pt[:, 0:512], start=first, stop=last,
                        skip_group_check=True,
                    )
                    nc.tensor.matmul(
                        cB[0:65, :],
                        lhsT=vp[:, kb, hB, :],
                        rhs=pt[:, 512:1024], start=first, stop=last,
                        skip_group_check=True,
                    )

                pipe = []
                for kb in range(SB):
                    pt = emit_scores_exp(kb)
                    if kb == 1 and pending_norm is not None:
                        emit_norm(*pending_norm)
                        pending_norm = None
                    if len(pipe) == 2:
                        emit_ctx(*pipe.pop(0))
                    if p == 0 and qw == 0:
                        # V one block ahead of its ctx deadline; the
                        # remaining k/q projection tiles land as full
                        # groups just before their first consumer (k-sc1
                        # at kb4, k-sc2 at kb8, k-sc3 at kb12, q-sc1 at
                        # window 1) — this window is PE-bound regardless
                        if kb + 1 < SB:
                            emit_proj_v(kb + 1)
                        full = {2: ("k", 1), 6: ("k", 2),
                                10: ("k", 3), 13: ("q", 1)}.get(kb)
                        if full:
                            emit_proj_qk_one(0, *full)
                    elif p == GC - 1 and qw >= 1:
                        if kb >= 2:
                            bg_step(3)
                    else:
                        bg_step(2)
                    pipe.append((kb, pt))
                for kb, pt in pipe:
                    emit_ctx(kb, pt)
                pending_norm = (p, q0, cA, cB)

        # ---- tail: last window's norm + out-projection ----
        # Four out-projection tiles accumulate concurrently across the
        # PSUM banks that are free by now (pproj, the two score slots, a
        # pctx slot), pair-major so the pair 0..2 contributions overlap
        # the last normalization instead of serializing behind it.
        bg_step(10 ** 9)  # drain any spilled background work
        # The last norm's bc tile is allocated before any po tiles (its
        # pscore slot must not rotate onto a live accumulator), but its
        # broadcast matmuls are emitted after the first round of pair-0
        # out-projection matmuls so the PE never idles on the DVE den
        # copies; the g=3 matmuls then land roughly when the norm's
        # lane-shift DMA completes.
        pn_p, pn_q0, pn_cA, pn_cB = pending_norm
        pending_norm = None
        den = norm.tile([P, 1024], BF16, tag="den")
        nc.vector.tensor_copy(den[64:65, 0:512], pn_cA[64:65, :])
        nc.vector.tensor_copy(den[64:65, 512:1024], pn_cB[64:65, :])
        bc = pscore.tile([P, 1024], FP32, tag="pscore", name="bc")
        w3 = (NW - 1) * W
        norm_done = False
        for group, size in ((0, 3), (1, 3), (2, 2)):
            tiles = []
            for idx in range(size):
                t = group * 3 + idx
                sb, nck = divmod(t, 2)
                pool, tag = ((pproj, "proj"), (pscore, "pscore"),
                             (pctx, "pctx"))[idx]
                po = pool.tile([P, 512], FP32, tag=tag, name="po")
                tiles.append((po, w3 + sb * P, nck))
            for g in range(GC):
                for po, row, nck in tiles:
                    nc.tensor.matmul(
                        po[:],
                        lhsT=ctxT[:, g, row:row + P],
                        rhs=wo_sb[:, g, nck * 512:(nck + 1) * 512],
                        start=(g == 0),
                        stop=(g == GC - 1),
                    )
                if not norm_done:
                    norm_done = True
                    nc.tensor.matmul(
                        bc[0:64, 0:512], lhsT=ones_sb[64:65, :],
                        rhs=den[64:65, 0:512], start=True, stop=True,
                    )
                    nc.tensor.matmul(
                        bc[0:64, 512:1024], lhsT=ones_sb[64:65, :],
                        rhs=den[64:65, 512:1024], start=True, stop=True,
                    )
                    rec = norm.tile([P, 1024], FP32, tag="rec")
                    nc.vector.reciprocal_approx_fast(
                        rec[0:64, 0:512], bc[0:64, 0:512]
                    )
                    nc.vector.tensor_tensor(
                        ctxT[0:64, pn_p, pn_q0:pn_q0 + 512],
                        pn_cA[0:64, :],
                        rec[0:64, 0:512],
                        mybir.AluOpType.mult,
                    )
                    nc.vector.reciprocal_approx_fast(
                        rec[0:64, 512:1024], bc[0:64, 512:1024]
                    )
                    tmpB = norm.tile([P, 512], BF16, tag="tmpB")
                    nc.vector.tensor_tensor(
                        tmpB[0:64, :],
                        pn_cB[0:64, :],
                        rec[0:64, 512:1024],
                        mybir.AluOpType.mult,
                    )
                    nc.sync.dma_start(
                        ctxT[64:128, pn_p, pn_q0:pn_q0 + 512], tmpB[0:64, :]
                    )
            for po, row, nck in tiles:
                ob = outp.tile([P, 512], FP32, tag="ob")
                nc.vector.tensor_copy(ob[:], po[:])
                nc.sync.dma_start(
                    out_d[row:row + P, nck * 512:(nck + 1) * 512], ob[:]
                )

    nc.compile()
    _BUILD_CACHE[key] = nc
    return nc


def make_shards(x, Wq, Wk, Wv, Wo):
    """Split full inputs into 8 per-core input maps.

    Host-side layout prep only (dtype narrowing + transpose): the kernel
    consumes bf16 and x with the model dim on partitions.
    """
    import ml_dtypes
    BF = ml_dtypes.bfloat16
    x = np.asarray(x, dtype=np.float32)
    xt = np.ascontiguousarray(x.transpose(0, 2, 1)).astype(BF)  # [B, D, S]
    Wqb = np.asarray(Wq, dtype=np.float32).astype(BF)
    Wkb = np.asarray(Wk, dtype=np.float32).astype(BF)
    Wvb = np.asarray(Wv, dtype=np.float32).astype(BF)
    Wob = np.asarray(Wo, dtype=np.float32).astype(BF)
    shards = []
    for c in range(NCORES):
        b, g = divmod(c, 2)
        cs = slice(g * G, (g + 1) * G)
        shards.append({
            "xt": xt[b],
            "wq": np.ascontiguousarray(Wqb[:, cs]),
            "wk": np.ascontiguousarray(Wkb[:, cs]),
            "wv": np.ascontiguousarray(Wvb[:, cs]),
            "wo": np.ascontiguousarray(Wob[cs, :]),
        })
    return shards


def combine(results, bo):
    """Sum head-group partials per batch and add bias."""
    bo = np.asarray(bo, dtype=np.float32)
    outs = [results[c]["out"] for c in range(NCORES)]
    return np.stack([outs[2 * b] + outs[2 * b + 1] for b in range(B)]) + bo


def run_shards(shards, trace=False, **kw):
    from concourse.bass_utils import run_bass_kernel_spmd
    nc = build_mha()
    return run_bass_kernel_spmd(nc, shards, list(range(NCORES)), trace=trace, **kw)


def kernel(x, Wq, Wk, Wv, Wo, bo):
    res = run_shards(make_shards(x, Wq, Wk, Wv, Wo))
    return combine(res.results, bo)
